# revision 4
# baseline (speedup 1.0000x reference)
"""Trainium2 Bass kernel for nn_DenseBayesian (dense + hard LWTA grouped argmax mask).

Computes out = x @ W.T + b, then per group of U=4 output units keeps only the
argmax unit (others zeroed). Data-parallel over 8 NeuronCores along the row axis.

Matmul numerics (variant "a"): fp16x3 split product (x = xh + xl, W.T = wh + wl
in fp16; out = xl@wh + xh@wl + xh@wh accumulated in fp32 PSUM) - ~22 effective
mantissa bits at 1 cycle/row.

Matmul numerics (variant "b"): fp16 main product xh@wh plus fp8 (e4m3)
DoubleRow correction products (xl*2^11)@(wh*2^6) + xh@(wl*2^17) accumulated in
a second PSUM at scale 2^17. The scalar engine rescales the correction to fp16
and the PE folds it into the main PSUM with an identity matmul. Same ~2^-15
effective precision class on the logits, ~1.5x fewer PE cycles.

LWTA masking: instead of materializing the dense masked [N, 512] output (3 full
DVE passes + 64MB/core of output DMA), the winner index is packed into the low
2 mantissa bits of the f32 logit on-chip:

    v = (bits(u) & ~3) | lane_id        (one fused scalar_tensor_tensor pass)
    m = reduce_max(v, per group of 4)   (one reduce pass)

A single f32 per group then carries both the winner value (error <= 3 ulps) and
its index; the host unpacks the bits and scatters into the dense f32 output.
Output DMA drops 4x to 16MB/core.

Self-contained: hardcodes the problem shapes; only needs numpy + the concourse
runtime available on the host.
"""
import numpy as np

import concourse.bass as bass
import concourse.mybir as mybir
import concourse.tile as tile
from concourse import bacc
from concourse.bass_utils import run_bass_kernel_spmd

f32 = mybir.dt.float32
f16 = mybir.dt.float16
f8 = mybir.dt.float8e4
u32 = mybir.dt.uint32

N = 262144
DIN = 256
DOUT = 512
U = 4
NCORES = 8
ROWS = N // NCORES          # 32768 rows per core
MACRO = 256                 # rows per macro-tile (2 psum banks of 128 rows)
P = 128
KC = DIN // P               # k chunks
G = DOUT // U               # groups per row (128)

SXL = 2.0 ** 11             # xl fp8 pre-scale
SWH = 2.0 ** 6              # wh fp8 pre-scale
SWL = 2.0 ** 17             # wl fp8 pre-scale
SCORR = 2.0 ** -17          # correction PSUM scale (1 / (SXL * SWH))

DR = mybir.MatmulPerfMode.DoubleRow


def _mask_and_store(nc, vpool, mpool, it, msk, ps, pk_dst, mask_engine):
    """v = (bits(u) & ~3) | lane_id; m2 = grouped max; DMA out."""
    eng = nc.gpsimd if mask_engine == "pool" else nc.vector
    v = vpool.tile([P, 2 * DOUT], f32)
    eng.scalar_tensor_tensor(
        v[:].bitcast(u32), ps[:].bitcast(u32), msk[:, 0:1], it[:],
        op0=mybir.AluOpType.bitwise_and,
        op1=mybir.AluOpType.bitwise_or)
    m2 = mpool.tile([P, 2 * G], f32)
    nc.vector.tensor_reduce(
        m2[:], v[:].rearrange("p (g s) -> p g s", s=U),
        axis=mybir.AxisListType.X, op=mybir.AluOpType.max)
    nc.sync.dma_start(pk_dst, m2[:].rearrange("p (s g) -> p s g", s=2))


def build_program(n_macros: int, with_bias: bool, variant: str = "a"):
    """One NeuronCore program: n_macros macro-tiles of 256 rows each."""
    nc = bacc.Bacc("TRN2", target_bir_lowering=False)
    mask_engine = "pool" if variant in ("b", "a_pool") else "dve"

    wh_d = nc.dram_tensor("wh", [P, KC, DOUT], f16, kind="ExternalInput")
    if variant.startswith("a"):
        xh_d = nc.dram_tensor("xh", [n_macros, P, KC, MACRO], f16, kind="ExternalInput")
        xl_d = nc.dram_tensor("xl", [n_macros, P, KC, MACRO], f16, kind="ExternalInput")
        wl_d = nc.dram_tensor("wl", [P, KC, DOUT], f16, kind="ExternalInput")
    else:
        xh_d = nc.dram_tensor("xh", [n_macros, P, KC, MACRO], f16, kind="ExternalInput")
        xh8_d = nc.dram_tensor("xh8", [n_macros, P, KC, MACRO], f8, kind="ExternalInput")
        xl8_d = nc.dram_tensor("xl8", [n_macros, P, KC, MACRO], f8, kind="ExternalInput")
        wh8_d = nc.dram_tensor("wh8", [P, KC, DOUT], f8, kind="ExternalInput")
        wl8_d = nc.dram_tensor("wl8", [P, KC, DOUT], f8, kind="ExternalInput")
    if with_bias:
        bh_d = nc.dram_tensor("bh", [1, DOUT], f16, kind="ExternalInput")
        bl_d = nc.dram_tensor("bl", [1, DOUT], f16, kind="ExternalInput")
    # packed winner (value with idx in low 2 bits), row = mt*256 + s*128 + p
    pk_d = nc.dram_tensor("pk", [n_macros, P, 2, G], f32, kind="ExternalOutput")

    with tile.TileContext(nc) as tc:
        with tc.tile_pool(name="wpool", bufs=1) as wpool, \
             tc.tile_pool(name="xpool", bufs=4) as xpool, \
             tc.tile_pool(name="cspool", bufs=3) as cspool, \
             tc.tile_pool(name="vpool", bufs=3) as vpool, \
             tc.tile_pool(name="mpool", bufs=3) as mpool, \
             tc.tile_pool(name="psm", bufs=2, space="PSUM") as psm, \
             tc.tile_pool(name="psc", bufs=2, space="PSUM") as psc:

            wh = wpool.tile([P, KC, DOUT], f16)
            nc.sync.dma_start(wh[:], wh_d[:])
            if variant.startswith("a"):
                wl = wpool.tile([P, KC, DOUT], f16)
                nc.sync.dma_start(wl[:], wl_d[:])
            else:
                wh8 = wpool.tile([P, KC, DOUT], f8)
                nc.sync.dma_start(wh8[:], wh8_d[:])
                wl8 = wpool.tile([P, KC, DOUT], f8)
                nc.sync.dma_start(wl8[:], wl8_d[:])
                # fp16 identity for folding the correction PSUM into main
                itcol = wpool.tile([P, P], u32)
                nc.gpsimd.iota(itcol[:], pattern=[[1, P]], base=0,
                               channel_multiplier=0)
                rowid = wpool.tile([P, 1], u32)
                nc.gpsimd.iota(rowid[:], pattern=[[0, 1]], base=0,
                               channel_multiplier=1)
                ident = wpool.tile([P, P], f16)
                nc.vector.tensor_scalar(ident[:], itcol[:], rowid[:, 0:1], None,
                                        op0=mybir.AluOpType.is_equal)
            if with_bias:
                bh = wpool.tile([1, DOUT], f16)
                nc.sync.dma_start(bh[:], bh_d[:])
                bl = wpool.tile([1, DOUT], f16)
                nc.sync.dma_start(bl[:], bl_d[:])
                ones = wpool.tile([1, P], f16)
                nc.gpsimd.memset(ones[:], 1.0)

            # lane-id pattern 0,1,2,3 repeating + the ~3 AND-mask scalar
            it = wpool.tile([P, 2 * DOUT], u32)
            nc.gpsimd.iota(it[:], pattern=[[0, 2 * G], [1, U]], base=0,
                           channel_multiplier=0)
            msk = wpool.tile([P, 1], u32)
            nc.vector.memset(msk[:], 0xFFFFFFFC)

            for mt in range(n_macros):
                if variant.startswith("a"):
                    xh_t = xpool.tile([P, KC, MACRO], f16, tag="xh")
                    nc.sync.dma_start(xh_t[:], xh_d[mt, :, :, :])
                    xl_t = xpool.tile([P, KC, MACRO], f16, tag="xl")
                    nc.sync.dma_start(xl_t[:], xl_d[mt, :, :, :])

                    ps = psm.tile([P, 2 * DOUT], f32)
                    for s in range(2):
                        acc = ps[:, s * DOUT:(s + 1) * DOUT]
                        mms = []
                        if with_bias:
                            mms.append((ones[:, :], bh[:, :]))
                            mms.append((ones[:, :], bl[:, :]))
                        rs = slice(s * P, (s + 1) * P)
                        for (xa, wb) in ((xl_t, wh), (xh_t, wl), (xh_t, wh)):
                            for c in range(KC):
                                mms.append((xa[:, c, rs], wb[:, c, :]))
                        last = len(mms) - 1
                        for i, (lhsT, rhs) in enumerate(mms):
                            nc.tensor.matmul(acc, lhsT, rhs,
                                             start=(i == 0), stop=(i == last))
                else:
                    xh_t = xpool.tile([P, KC, MACRO], f16, tag="xh")
                    nc.sync.dma_start(xh_t[:], xh_d[mt, :, :, :])
                    xh8_t = xpool.tile([P, KC, MACRO], f8, tag="xh8")
                    nc.sync.dma_start(xh8_t[:], xh8_d[mt, :, :, :])
                    xl8_t = xpool.tile([P, KC, MACRO], f8, tag="xl8")
                    nc.sync.dma_start(xl8_t[:], xl8_d[mt, :, :, :])

                    cps = psc.tile([P, 2 * DOUT], f32)
                    ps = psm.tile([P, 2 * DOUT], f32)
                    # fp8 DoubleRow corrections first so the scalar engine can
                    # rescale them while the PE runs the fp16 main products
                    for s in range(2):
                        rs = slice(s * P, (s + 1) * P)
                        cacc = cps[:, s * DOUT:(s + 1) * DOUT]
                        nc.tensor.matmul(cacc, xl8_t[:, :, rs], wh8[:, :, :],
                                         start=True, stop=False, perf_mode=DR)
                        nc.tensor.matmul(cacc, xh8_t[:, :, rs], wl8[:, :, :],
                                         start=False, stop=True, perf_mode=DR)
                    csb = cspool.tile([P, 2, DOUT], f16)
                    for s in range(2):
                        nc.scalar.activation(
                            csb[:, s, :], cps[:, s * DOUT:(s + 1) * DOUT],
                            mybir.ActivationFunctionType.Copy, scale=SCORR)
                    for s in range(2):
                        rs = slice(s * P, (s + 1) * P)
                        acc = ps[:, s * DOUT:(s + 1) * DOUT]
                        first = True
                        if with_bias:
                            nc.tensor.matmul(acc, ones[:, :], bh[:, :],
                                             start=True, stop=False)
                            nc.tensor.matmul(acc, ones[:, :], bl[:, :],
                                             start=False, stop=False)
                            first = False
                        for c in range(KC):
                            nc.tensor.matmul(acc, xh_t[:, c, rs], wh[:, c, :],
                                             start=first, stop=False)
                            first = False
                    for s in range(2):
                        acc = ps[:, s * DOUT:(s + 1) * DOUT]
                        nc.tensor.matmul(acc, ident[:], csb[:, s, :],
                                         start=False, stop=True)

                _mask_and_store(nc, vpool, mpool, it, msk, ps, pk_d[mt],
                                mask_engine)

    nc.compile()
    return nc


_programs: dict = {}


def _get_program(n_macros: int, with_bias: bool, variant: str = "a"):
    key = (n_macros, with_bias, variant)
    if key not in _programs:
        _programs[key] = build_program(n_macros, with_bias, variant)
    return _programs[key]


def _split_fp16(a: np.ndarray):
    hi = a.astype(np.float16)
    lo = (a - hi.astype(np.float32)).astype(np.float16)
    return hi, lo


def _pack_b(b: np.ndarray):
    """[DOUT] fp32 -> (hi, lo) [1, DOUT] fp16."""
    return _split_fp16(np.ascontiguousarray(b.astype(np.float32).reshape(1, DOUT)))


def _tile_x(a: np.ndarray, n_macros: int) -> np.ndarray:
    """[rows, DIN] -> [n_macros, P, KC, MACRO] keeping dtype."""
    at = np.ascontiguousarray(a.T)                      # [DIN, rows]
    at = at.reshape(KC, P, n_macros, MACRO)             # [c, p, mt, r]
    return np.ascontiguousarray(at.transpose(2, 1, 0, 3))


def _pack_x(xs: np.ndarray, n_macros: int):
    """[rows, DIN] fp32 -> (hi, lo) tiled [n_macros, P, KC, MACRO] fp16."""
    hi, lo = _split_fp16(xs)
    return [_tile_x(a, n_macros) for a in (hi, lo)]


def _pack_x8(xs: np.ndarray, n_macros: int):
    """fp32 rows -> (xh fp16, xh8 fp8, xl8 fp8 scaled) tiles for variant b."""
    f8np = mybir.dt.np(f8)
    hi = xs.astype(np.float16)
    lo32 = xs - hi.astype(np.float32)
    xh = _tile_x(hi, n_macros)
    xh8 = _tile_x(hi.astype(f8np), n_macros)
    xl8 = _tile_x((lo32 * SXL).astype(f8np), n_macros)
    return xh, xh8, xl8


def _tile_w(a: np.ndarray) -> np.ndarray:
    """[DIN, DOUT] -> [P, KC, DOUT] keeping dtype."""
    return np.ascontiguousarray(a.reshape(KC, P, DOUT).transpose(1, 0, 2))


def _pack_w(W: np.ndarray):
    """[DOUT, DIN] fp32 -> (hi, lo) tiled [P, KC, DOUT] fp16 of W.T."""
    wT = W.astype(np.float32).T                         # [DIN, DOUT]
    hi, lo = _split_fp16(np.ascontiguousarray(wT))
    return [_tile_w(a) for a in (hi, lo)]


def _pack_w8(W: np.ndarray):
    """-> (wh fp16, wh8 fp8 * 2^6, wl8 fp8 * 2^17) tiles for variant b."""
    f8np = mybir.dt.np(f8)
    wT = np.ascontiguousarray(W.astype(np.float32).T)   # [DIN, DOUT]
    hi = wT.astype(np.float16)
    lo32 = wT - hi.astype(np.float32)
    wh = _tile_w(hi)
    wh8 = _tile_w((wT * SWH).astype(f8np))
    wl8 = _tile_w((lo32 * SWL).astype(f8np))
    return wh, wh8, wl8


def _unpack_out(pk: np.ndarray, rows: int) -> np.ndarray:
    """[n_macros, P, 2, G] packed f32 -> dense [rows, DOUT] f32."""
    # row = mt*256 + s*128 + p  ->  [mt, s, p, g]
    m2 = np.ascontiguousarray(pk.transpose(0, 2, 1, 3)).reshape(rows, G)
    iv = m2.view(np.uint32)
    idx = (iv & np.uint32(3)).astype(np.int64)
    val = (iv & np.uint32(0xFFFFFFFC)).view(np.float32)
    out = np.zeros((rows, G, U), dtype=np.float32)
    np.put_along_axis(out, idx[:, :, None], val[:, :, None], axis=2)
    return out.reshape(rows, DOUT)


def _build_in_maps(x, W, b, with_bias, n_macros, variant):
    in_maps = []
    if variant.startswith("a"):
        wh, wl = _pack_w(W)
        for i in range(NCORES):
            xh, xl = _pack_x(x[i * ROWS:(i + 1) * ROWS], n_macros)
            im = {"xh": xh, "xl": xl, "wh": wh, "wl": wl}
            if with_bias:
                im["bh"], im["bl"] = _pack_b(b)
            in_maps.append(im)
    else:
        wh, wh8, wl8 = _pack_w8(W)
        for i in range(NCORES):
            xh, xh8, xl8 = _pack_x8(x[i * ROWS:(i + 1) * ROWS], n_macros)
            im = {"xh": xh, "xh8": xh8, "xl8": xl8,
                  "wh": wh, "wh8": wh8, "wl8": wl8}
            if with_bias:
                im["bh"], im["bl"] = _pack_b(b)
            in_maps.append(im)
    return in_maps


VARIANT = "a"


def kernel(x: np.ndarray, W: np.ndarray, b: np.ndarray) -> np.ndarray:
    x = np.asarray(x, dtype=np.float32)
    W = np.asarray(W, dtype=np.float32)
    b = np.asarray(b, dtype=np.float32)
    assert x.shape == (N, DIN) and W.shape == (DOUT, DIN) and b.shape == (DOUT,)

    with_bias = bool(np.any(b))
    n_macros = ROWS // MACRO
    nc = _get_program(n_macros, with_bias, VARIANT)

    in_maps = _build_in_maps(x, W, b, with_bias, n_macros, VARIANT)
    res = run_bass_kernel_spmd(nc, in_maps, list(range(NCORES)))
    return np.concatenate(
        [_unpack_out(res.results[i]["pk"], ROWS) for i in range(NCORES)], axis=0)
